# revision 25
# baseline (speedup 1.0000x reference)
"""Trainium2 Bass kernel for GWASEncoder (embedding_lookup).

Math: out[n] = (sum_t w[n,t] * proj(combined[n,t])) / max(sum_t w[n,t], 1e-8)
with proj linear -> pull the projection through the weighted sum:
  out[n] = sum_t (w*inv)[n,t] * P[token[n,t]]  +  M @ (q[n]*inv[n])
where P = trait_embed @ Wt.T (projected token table, gathered on device),
q[n] = [cat histogram (32), sum w*s, sum w], M = [Pc | Ws | b], inv = 1/max(sum w, eps).

Device work per core (data-parallel over nodes, tables replicated):
  dma_gather (SWDGE, bf16, lo/hi split tables for int16 idx range) of the
  projected rows, PE matmul-reduce (gathered chunk as lhsT, host-prebuilt
  sparse W-matrix streamed from HBM as rhs) accumulating into PSUM
  [128 d x 512 nodes]; PSUM -> SBUF copy -> DMA out in [d, node] layout
  (host transposes the final result).

Perf notes (HW-measured): the kernel sits at a gen/drain equilibrium of
~0.49 idx/ns aggregate (~503us stream for 245k idx):
  - SWDGE Q7 desc-gen: ~7.2ns/idx per queue core-pair unpacketed (8.4 with
    single_packet=True), 4 queues (HW max) in parallel => 0.56/ns capacity.
  - SDMA drain of 256B random-HBM descriptors: ~32.5ns/desc/engine x16
    => 0.49/ns; this throttles gen via ring await_space (calls inflate
    29->42us when 4 queues run). single_packet drains faster but gen is
    slower per idx; both configs land ~equal. 64-desc/engine packet limit
    caps packeted calls at 1024 idx.
Calls are host-assigned to queues by greedy least-loaded balancing
(round-robin left a 2x skew). num_idxs registers are hoisted (one per
distinct call size) - a fresh MOVE per call head-of-line-blocked the Pool
queue on a register hazard. W matrices are host-built (fp8e4m3, dup tokens
accumulated onto their deduped gather row) and DMA-streamed, so the DVE
never backpressures the gather; inv is folded into W and q so the output
needs no per-node scaling and no on-device transpose (host transposes).
Dead ends measured/analyzed: IndirectCopy (walrus rejects; per-core idx
work is 8x-replicated anyway), page-level dedup (PE scatter width),
vocab/token resharding (same idx count), fp8 table (256B desc granularity),
pair-descriptors (uniform ids don't pair), host-precomputed descriptors
(no API). Floor of this architecture ~= ramp + 245k/0.49ns + teardown.
"""

import sys

if "/opt/trn_rl_repo" not in sys.path:
    sys.path.insert(0, "/opt/trn_rl_repo")

import math

import ml_dtypes
import numpy as np

import concourse.bass as bass  # noqa: F401
import concourse.mybir as mybir
import concourse.tile as tile
from concourse import bacc
from concourse.bass_utils import run_bass_kernel_spmd
from concourse.library_config import mlp

bf16 = ml_dtypes.bfloat16
f8 = ml_dtypes.float8_e4m3fn

N, T, V, D = 30000, 64, 50000, 128
NCORES = 8
NPC = N // NCORES          # 3750 nodes per core
SPLIT = 32768              # int16 idx limit for dma_gather
PAGE = 512                 # psum bank columns (nodes per page)
GROUP = 64                 # node columns per rhs matmul
CALL_BIG = 32              # chunks per gather call (4096 idx)
NQUEUES = 4                # SWDGE queues (4 Q7 core pairs generate in parallel; HW max)
NPAGES = math.ceil(NPC / PAGE)
NS_PER_IDX = 6.2           # measured SWDGE desc-gen cost per index (per queue pair)
NS_PER_CALL = 900.0        # fixed per-call overhead


def _page_nodes(p):
    return min(PAGE, NPC - p * PAGE)


def _balance_perm(ids):
    """Per-core node permutation: rearrange nodes so each GROUP's lo-token
    count (ids < SPLIT) lands exactly on a multiple of 128. Then both
    ceil(lo/128) and ceil(hi/128) are exact and equal across cores, removing
    ~5-7% structural chunk padding (padding = pure Q7 descriptor-gen cost).
    """
    lo = (ids < SPLIT).sum(1).astype(np.int64)      # per node
    perm = np.arange(N)
    for c in range(NCORES):
        v = lo[c * NPC:(c + 1) * NPC]
        caps = [min(GROUP, NPC - g * GROUP) for g in range(math.ceil(NPC / GROUP))]
        ng = len(caps)
        order = np.argsort(-v, kind="stable")
        groups = [[] for _ in range(ng)]
        gi = 0
        for node in order:
            tries = 0
            while len(groups[gi % ng]) >= caps[gi % ng]:
                gi += 1
                tries += 1
                assert tries <= ng
            groups[gi % ng].append(int(node))
            gi += 1
        sums = np.array([v[g].sum() for g in [np.array(x) for x in groups]])
        # targets: per-group multiples of 128 via cumulative rounding
        cum = np.cumsum(sums)
        rcum = np.concatenate([[0], 128 * np.round(cum / 128.0)]).astype(np.int64)
        tgt = np.diff(rcum)
        tgt[-1] = sums.sum() - tgt[:-1].sum()       # absorber keeps total exact
        # swap repair: fix groups 0..ng-2 to hit targets exactly
        for g in range(ng - 1):
            delta = int(tgt[g] - sums[g])           # need sum[g] += delta
            guard = 0
            while delta != 0 and guard < 50:
                guard += 1
                best = None                          # (gain, h, i_idx, j_idx)
                for h in range(g + 1, ng):
                    va = v[np.array(groups[g])]
                    vb = v[np.array(groups[h])]
                    d = vb[None, :] - va[:, None]    # swap a<-b changes sum[g] by d
                    if delta > 0:
                        d = np.where(d <= delta, d, -10**9)
                    else:
                        d = np.where(d >= delta, d, 10**9)
                    ij = np.unravel_index(np.argmin(np.abs(d - delta)), d.shape)
                    gain = d[ij]
                    if abs(gain) > 0 and abs(gain - delta) < abs(delta):
                        if best is None or abs(gain - delta) < abs(best[0] - delta):
                            best = (gain, h, ij[0], ij[1])
                            if gain == delta:
                                break
                if best is None:
                    break
                gain, h, i_a, j_b = best
                a, b = groups[g][i_a], groups[h][j_b]
                groups[g][i_a], groups[h][j_b] = b, a
                sums[g] += gain
                sums[h] -= gain
                delta -= int(gain)
        pos = np.concatenate([np.array(g, dtype=np.int64) for g in groups])
        perm[c * NPC:(c + 1) * NPC] = c * NPC + pos
    return perm


def _split_equal(run, cap):
    """Split `run` chunks into equal-ish calls of at most `cap` chunks."""
    if run == 0:
        return []
    ncalls = math.ceil(run / cap)
    base = run // ncalls
    rem = run - base * ncalls
    return [base + (1 if i < rem else 0) for i in range(ncalls)]


def _prep(token_ids, scores, cat_ids, trait_embed, cat_embed, proj_w, proj_b):
    """Host-side: weights preprocessing + per-core stream packing."""
    ids = np.asarray(token_ids).astype(np.int64)
    scores = np.asarray(scores, dtype=np.float32)
    cats = np.asarray(cat_ids).astype(np.int64)
    perm = _balance_perm(ids)
    ids, scores, cats = ids[perm], scores[perm], cats[perm]
    trait_embed = np.asarray(trait_embed, dtype=np.float32)
    cat_embed = np.asarray(cat_embed, dtype=np.float32)
    proj_w = np.asarray(proj_w, dtype=np.float32)
    proj_b = np.asarray(proj_b, dtype=np.float32)

    Wt = proj_w[:, :D]           # [128, 128]
    Wc = proj_w[:, D:D + 8]      # [128, 8]
    Ws = proj_w[:, D + 8]        # [128]

    P = trait_embed @ Wt.T                      # [V, 128] projected table
    P_lo = np.ascontiguousarray(P[:SPLIT]).astype(bf16)
    P_hi = np.concatenate([np.zeros((1, D), np.float32), P[SPLIT:]], 0).astype(bf16)
    Pc = cat_embed @ Wc.T                       # [32, 128]
    MqT = np.concatenate([Pc, Ws[None, :], proj_b[None, :]], 0).astype(np.float32)  # [34,128]

    w = scores * (ids != 0)                     # [N, T]
    node_idx = np.repeat(np.arange(N, dtype=np.int64), T)
    hist = np.bincount(node_idx * 32 + cats.reshape(-1), weights=w.reshape(-1),
                       minlength=N * 32).reshape(N, 32)
    sws = (w * scores).sum(1)
    sw = w.sum(1)
    inv = (1.0 / np.maximum(sw, 1e-8)).astype(np.float32)
    q = np.concatenate([hist, sws[:, None], sw[:, None]], 1).astype(np.float32)  # [N,34]
    q *= inv[:, None]                           # fold the 1/sum_w into q
    wi = w * inv[:, None]                       # folded per-token weights

    # ---- structural chunk counts: max over cores per (page, group, table) ----
    lo_cnt = (ids < SPLIT).sum(1)               # per node (incl. id==0 pads -> lo)
    hi_cnt = T - lo_cnt
    ngroups = [math.ceil(_page_nodes(p) / GROUP) for p in range(NPAGES)]
    nchunks = []
    for p in range(NPAGES):
        per_t = [[], []]
        for g in range(ngroups[p]):
            n0 = p * PAGE + g * GROUP
            n1 = min(p * PAGE + _page_nodes(p), n0 + GROUP)
            best = [0, 0]
            for c in range(NCORES):
                sl = slice(c * NPC + n0, c * NPC + n1)
                idg = ids[sl].reshape(-1)
                ulo = len(np.unique(idg[idg < SPLIT]))
                uhi = len(np.unique(idg[idg >= SPLIT]))
                best[0] = max(best[0], math.ceil(ulo / 128))
                best[1] = max(best[1], math.ceil(uhi / 128))
            per_t[0].append(int(best[0]))
            per_t[1].append(int(best[1]))
        nchunks.append(per_t)

    # global chunk layout: page -> table -> group -> chunks
    chunk_group = []   # group index within page, per global chunk
    calls = []         # per page: list of [table, chunk0, nch, queue]
    last_chunk_of_page = []
    cbase = 0
    flat_calls = []
    for p in range(NPAGES):
        page_calls = []
        for t in (0, 1):
            run_chunks = sum(nchunks[p][t])
            for g in range(ngroups[p]):
                chunk_group.extend([g] * nchunks[p][t][g])
            if p == 0 and t == 0:
                # tiny warm-up calls, one per queue: desc-gen starts as soon
                # as a small idx slice lands while the bulk idx loads stream
                sizes = [2] * NQUEUES + _split_equal(run_chunks - 2 * NQUEUES,
                                                     CALL_BIG)
            elif p == NPAGES - 1:
                # finer calls at the end: queues taper together
                sizes = _split_equal(run_chunks, CALL_BIG // 2)
            else:
                sizes = _split_equal(run_chunks, CALL_BIG)
            done = 0
            for nch in sizes:
                page_calls.append([t, cbase + done, nch, 0])
                flat_calls.append(page_calls[-1])
                done += nch
            cbase += run_chunks
        calls.append(page_calls)
        last_chunk_of_page.append(cbase - 1)
    total_chunks = cbase

    # greedy least-loaded queue assignment (issue order fixed)
    qload = [0.0] * NQUEUES
    for call in flat_calls:
        dur = call[2] * 128 * NS_PER_IDX + NS_PER_CALL
        qmin = int(np.argmin(qload))
        call[3] = qmin
        qload[qmin] += dur

    meta = dict(calls=calls, chunk_group=chunk_group,
                last_chunk_of_page=last_chunk_of_page,
                total_chunks=total_chunks, ngroups=ngroups, perm=perm,
                qload=qload)

    # ---- per-core stream arrays ----
    in_maps = []
    for c in range(NCORES):
        idx_flat = np.zeros(total_chunks * 128, np.int16)
        tok_row, tok_col, tok_w = [], [], []
        cb = 0
        for p in range(NPAGES):
            for t in (0, 1):
                for g in range(ngroups[p]):
                    n0 = p * PAGE + g * GROUP
                    n1 = min(p * PAGE + _page_nodes(p), n0 + GROUP)
                    sl = slice(c * NPC + n0, c * NPC + n1)
                    idg = ids[sl]          # [ng, T]
                    wg = wi[sl]
                    m = (idg < SPLIT) if t == 0 else (idg >= SPLIT)
                    rows, cols = np.nonzero(m)
                    vals = idg[rows, cols]
                    if t == 1:
                        vals = vals - SPLIT + 1
                    # dedup: gather each unique id once; W carries multiplicity
                    uniq, inv_u = np.unique(vals, return_inverse=True)
                    k = len(uniq)
                    nch = nchunks[p][t][g]
                    off = cb * 128
                    idx_flat[off:off + k] = uniq.astype(np.int16)
                    tok_row.append(off + inv_u)
                    tok_col.append(rows)
                    tok_w.append(wg[rows, cols])
                    cb += nch
        assert cb == total_chunks

        # idx pack: per call [16, cols] tiled to 128 partitions
        idx_cols = np.empty((128, total_chunks * 8), np.int16)
        for page_calls in calls:
            for (_, c0, nch, _q) in page_calls:
                fl = idx_flat[c0 * 128:(c0 + nch) * 128]
                blk = fl.reshape(-1, 16).T           # [16, nch*8]
                idx_cols[:, c0 * 8:(c0 + nch) * 8] = np.tile(blk, (8, 1))

        # host-built W stream: [128, TC*64] fp8, chunk-major, with dup
        # tokens accumulated onto their unique gathered row
        wmat = np.zeros((total_chunks, 128, GROUP), np.float32)
        trow = np.concatenate(tok_row)
        tcol = np.concatenate(tok_col)
        tw = np.concatenate(tok_w)
        np.add.at(wmat, (trow // 128, trow % 128, tcol), tw)
        wmat = np.ascontiguousarray(
            wmat.transpose(1, 0, 2).reshape(128, total_chunks * GROUP)).astype(f8)

        qc = np.zeros((NPAGES * PAGE, 34), np.float32)
        qc[:NPC] = q[c * NPC:(c + 1) * NPC]
        q_arr = np.ascontiguousarray(qc.T)           # [34, NPAGES*PAGE]

        in_maps.append({
            "p_lo": np.asarray(P_lo), "p_hi": np.asarray(P_hi),
            "idxs": idx_cols, "wmat": wmat, "q": q_arr, "mqt": MqT,
        })
    return meta, in_maps


def _build(meta):
    f32, bft, i16 = mybir.dt.float32, mybir.dt.bfloat16, mybir.dt.int16
    TC = meta["total_chunks"]
    calls, chunk_group = meta["calls"], meta["chunk_group"]
    last_of = meta["last_chunk_of_page"]

    nc = bacc.Bacc("TRN2", target_bir_lowering=False, debug=False,
                   num_swdge_queues=NQUEUES)
    p_lo_d = nc.dram_tensor("p_lo", [SPLIT, D], bft, kind="ExternalInput")
    p_hi_d = nc.dram_tensor("p_hi", [V - SPLIT + 1, D], bft, kind="ExternalInput")
    idx_d = nc.dram_tensor("idxs", [128, TC * 8], i16, kind="ExternalInput")
    w_d = nc.dram_tensor("wmat", [128, TC * GROUP], mybir.dt.float8e4,
                         kind="ExternalInput")
    q_d = nc.dram_tensor("q", [34, NPAGES * PAGE], f32, kind="ExternalInput")
    mqt_d = nc.dram_tensor("mqt", [34, D], f32, kind="ExternalInput")
    out_d = nc.dram_tensor("out", [128, NPAGES * PAGE], f32, kind="ExternalOutput")

    with tile.TileContext(nc) as tc:
        with (
            tc.tile_pool(name="const", bufs=1) as const,
            tc.tile_pool(name="gpb", bufs=10) as gpb,
            tc.tile_pool(name="wb", bufs=8) as wbp,
            tc.tile_pool(name="nsb", bufs=2) as nsb,
            tc.tile_pool(name="psm", bufs=2, space="PSUM") as psm,
        ):
            idx_sb = const.tile([128, TC * 8], i16)
            q_sb = const.tile([34, NPAGES * PAGE], f32)
            mqt_sb = const.tile([34, D], f32)

            # idx loads: the warm-up calls' slice alone (fast ramp), then the
            # rest of page 0, then per page.
            nc.gpsimd.load_library(mlp)
            nch_0 = sum(cl[2] for cl in calls[0][:NQUEUES])
            nc.sync.dma_start(idx_sb[:, :nch_0 * 8], idx_d[:, :nch_0 * 8])
            cprev = nch_0
            for p in range(NPAGES):
                cend = last_of[p] + 1
                if cend > cprev:
                    nc.sync.dma_start(idx_sb[:, cprev * 8:cend * 8],
                                      idx_d[:, cprev * 8:cend * 8])
                cprev = cend
            nc.sync.dma_start(q_sb[:], q_d[:])
            nc.sync.dma_start(mqt_sb[:], mqt_d[:])

            size_regs = {}
            for p in range(NPAGES):
                ps = psm.tile([128, PAGE], mybir.dt.float32)
                nc.tensor.matmul(ps[:], mqt_sb[:],
                                 q_sb[:, p * PAGE:(p + 1) * PAGE],
                                 start=True, stop=False)
                for (t, c0, nch, qn) in calls[p]:
                    src = p_lo_d if t == 0 else p_hi_d
                    g_t = gpb.tile([128, CALL_BIG, D], bft, tag="gb")
                    if nch * 128 not in size_regs:
                        size_regs[nch * 128] = nc.gpsimd.to_reg(nch * 128)
                    nc.gpsimd.dma_gather(
                        g_t[:, :nch, :], src[:],
                        idx_sb[:, c0 * 8:(c0 + nch) * 8],
                        nch * 128, size_regs[nch * 128], D, queue_num=qn,
                        single_packet=False)
                    w_t = wbp.tile([128, CALL_BIG * GROUP], mybir.dt.float8e4,
                                   tag="wb")
                    nc.sync.dma_start(w_t[:, :nch * GROUP],
                                      w_d[:, c0 * GROUP:(c0 + nch) * GROUP])
                    for k in range(nch):
                        c = c0 + k
                        g = chunk_group[c]
                        nc.tensor.matmul(
                            ps[:, g * GROUP:(g + 1) * GROUP],
                            g_t[:, k, :], w_t[:, k * GROUP:(k + 1) * GROUP],
                            start=False, stop=(c == last_of[p]))

                num_sb = nsb.tile([128, PAGE], mybir.dt.float32)
                nc.vector.tensor_copy(num_sb[:], ps[:])
                nc.sync.dma_start(out_d[:, p * PAGE:(p + 1) * PAGE], num_sb[:])

    nc.compile()
    return nc


TRACE = False       # test harness can flip this for profiling
LAST_RESULT = None  # BassKernelResults of the most recent run


def kernel(**inputs) -> np.ndarray:
    global LAST_RESULT
    meta, in_maps = _prep(**inputs)
    nc = _build(meta)
    res = run_bass_kernel_spmd(nc, in_maps, list(range(NCORES)), trace=TRACE)
    LAST_RESULT = res
    outs = [np.asarray(r["out"]).T[:NPC] for r in res.results]
    rows = np.concatenate(outs, 0).astype(np.float32)
    full = np.empty_like(rows)
    full[meta["perm"]] = rows        # undo the node-balancing permutation
    return full


if __name__ == "__main__":
    rng = np.random.default_rng(0)
    demo = dict(
        token_ids=rng.integers(0, V, (N, T)),
        scores=rng.random((N, T), dtype=np.float32),
        cat_ids=rng.integers(0, 32, (N, T)),
        trait_embed=(rng.standard_normal((V, D)).astype(np.float32) * 0.02),
        cat_embed=(rng.standard_normal((32, 8)).astype(np.float32) * 0.02),
        proj_w=rng.standard_normal((D, D + 9)).astype(np.float32) / np.sqrt(137),
        proj_b=np.zeros(D, np.float32),
    )
    demo["trait_embed"][0] = 0
    out = kernel(**demo)
    print(out.shape, out.dtype)


# revision 27
# speedup vs baseline: 1.1616x; 1.1616x over previous
"""Trainium2 Bass kernel for GWASEncoder (embedding_lookup).

Math: out[n] = (sum_t w[n,t] * proj(combined[n,t])) / max(sum_t w[n,t], 1e-8)
with proj linear -> pull the projection through the weighted sum:
  out[n] = sum_t (w*inv)[n,t] * P[token[n,t]]  +  M @ (q[n]*inv[n])
where P = trait_embed @ Wt.T (projected token table, gathered on device),
q[n] = [cat histogram (32), sum w*s, sum w], M = [Pc | Ws | b], inv = 1/max(sum w, eps).

Device work per core (data-parallel over nodes, tables replicated):
  dma_gather (SWDGE, bf16, lo/hi split tables for int16 idx range) of the
  projected rows, PE matmul-reduce (gathered chunk as lhsT, host-prebuilt
  sparse W-matrix streamed from HBM as rhs) accumulating into PSUM
  [128 d x 512 nodes]; PSUM -> SBUF copy -> DMA out in [d, node] layout
  (host transposes the final result).

Perf notes (HW-measured): the kernel sits at a gen/drain equilibrium of
~0.49 idx/ns aggregate (~503us stream for 245k idx):
  - SWDGE Q7 desc-gen: ~7.2ns/idx per queue core-pair unpacketed (8.4 with
    single_packet=True), 4 queues (HW max) in parallel => 0.56/ns capacity.
  - SDMA drain of 256B random-HBM descriptors: ~32.5ns/desc/engine x16
    => 0.49/ns; this throttles gen via ring await_space (calls inflate
    29->42us when 4 queues run). single_packet drains faster but gen is
    slower per idx; both configs land ~equal. 64-desc/engine packet limit
    caps packeted calls at 1024 idx.
Calls are host-assigned to queues by greedy least-loaded balancing
(round-robin left a 2x skew). num_idxs registers are hoisted (one per
distinct call size) - a fresh MOVE per call head-of-line-blocked the Pool
queue on a register hazard. W matrices are host-built (fp8e4m3, dup tokens
accumulated onto their deduped gather row) and DMA-streamed, so the DVE
never backpressures the gather; inv is folded into W and q so the output
needs no per-node scaling and no on-device transpose (host transposes).
Dead ends measured/analyzed: IndirectCopy (walrus rejects; per-core idx
work is 8x-replicated anyway), page-level dedup (PE scatter width),
vocab/token resharding (same idx count), fp8 table (256B desc granularity),
pair-descriptors (uniform ids don't pair), host-precomputed descriptors
(no API). Floor of this architecture ~= ramp + 245k/0.49ns + teardown.
"""

import sys

if "/opt/trn_rl_repo" not in sys.path:
    sys.path.insert(0, "/opt/trn_rl_repo")

import math

import ml_dtypes
import numpy as np

import concourse.bass as bass  # noqa: F401
import concourse.mybir as mybir
import concourse.tile as tile
from concourse import bacc
from concourse.bass_utils import run_bass_kernel_spmd
from concourse.library_config import mlp

bf16 = ml_dtypes.bfloat16
f8 = ml_dtypes.float8_e4m3fn

N, T, V, D = 30000, 64, 50000, 128
NCORES = 8
NPC = N // NCORES          # 3750 nodes per core
SPLIT = 32768              # int16 idx limit for dma_gather
PAGE = 512                 # psum bank columns (nodes per page)
GROUP = 64                 # node columns per rhs matmul
CALL_BIG = 32              # chunks per gather call (4096 idx)
NQUEUES = 4                # SWDGE queues (4 Q7 core pairs generate in parallel; HW max)
NPAGES = math.ceil(NPC / PAGE)
NS_PER_IDX = 6.2           # measured SWDGE desc-gen cost per index (per queue pair)
NS_PER_CALL = 900.0        # fixed per-call overhead


def _page_nodes(p):
    return min(PAGE, NPC - p * PAGE)


def _balance_perm(ids):
    """Per-core node permutation: rearrange nodes so each GROUP's lo-token
    count (ids < SPLIT) lands exactly on a multiple of 128. Then both
    ceil(lo/128) and ceil(hi/128) are exact and equal across cores, removing
    ~5-7% structural chunk padding (padding = pure Q7 descriptor-gen cost).
    """
    lo = (ids < SPLIT).sum(1).astype(np.int64)      # per node
    perm = np.arange(N)
    for c in range(NCORES):
        v = lo[c * NPC:(c + 1) * NPC]
        caps = [min(GROUP, NPC - g * GROUP) for g in range(math.ceil(NPC / GROUP))]
        ng = len(caps)
        order = np.argsort(-v, kind="stable")
        groups = [[] for _ in range(ng)]
        gi = 0
        for node in order:
            tries = 0
            while len(groups[gi % ng]) >= caps[gi % ng]:
                gi += 1
                tries += 1
                assert tries <= ng
            groups[gi % ng].append(int(node))
            gi += 1
        sums = np.array([v[g].sum() for g in [np.array(x) for x in groups]])
        # targets: per-group multiples of 128 via cumulative rounding
        cum = np.cumsum(sums)
        rcum = np.concatenate([[0], 128 * np.round(cum / 128.0)]).astype(np.int64)
        tgt = np.diff(rcum)
        tgt[-1] = sums.sum() - tgt[:-1].sum()       # absorber keeps total exact
        # swap repair: fix groups 0..ng-2 to hit targets exactly
        for g in range(ng - 1):
            delta = int(tgt[g] - sums[g])           # need sum[g] += delta
            guard = 0
            while delta != 0 and guard < 50:
                guard += 1
                best = None                          # (gain, h, i_idx, j_idx)
                for h in range(g + 1, ng):
                    va = v[np.array(groups[g])]
                    vb = v[np.array(groups[h])]
                    d = vb[None, :] - va[:, None]    # swap a<-b changes sum[g] by d
                    if delta > 0:
                        d = np.where(d <= delta, d, -10**9)
                    else:
                        d = np.where(d >= delta, d, 10**9)
                    ij = np.unravel_index(np.argmin(np.abs(d - delta)), d.shape)
                    gain = d[ij]
                    if abs(gain) > 0 and abs(gain - delta) < abs(delta):
                        if best is None or abs(gain - delta) < abs(best[0] - delta):
                            best = (gain, h, ij[0], ij[1])
                            if gain == delta:
                                break
                if best is None:
                    break
                gain, h, i_a, j_b = best
                a, b = groups[g][i_a], groups[h][j_b]
                groups[g][i_a], groups[h][j_b] = b, a
                sums[g] += gain
                sums[h] -= gain
                delta -= int(gain)
        pos = np.concatenate([np.array(g, dtype=np.int64) for g in groups])
        perm[c * NPC:(c + 1) * NPC] = c * NPC + pos
    return perm


def _split_equal(run, cap):
    """Split `run` chunks into equal-ish calls of at most `cap` chunks."""
    if run == 0:
        return []
    ncalls = math.ceil(run / cap)
    base = run // ncalls
    rem = run - base * ncalls
    return [base + (1 if i < rem else 0) for i in range(ncalls)]


def _prep(token_ids, scores, cat_ids, trait_embed, cat_embed, proj_w, proj_b):
    """Host-side: weights preprocessing + per-core stream packing."""
    ids = np.asarray(token_ids).astype(np.int64)
    scores = np.asarray(scores, dtype=np.float32)
    cats = np.asarray(cat_ids).astype(np.int64)
    perm = _balance_perm(ids)
    ids, scores, cats = ids[perm], scores[perm], cats[perm]
    trait_embed = np.asarray(trait_embed, dtype=np.float32)
    cat_embed = np.asarray(cat_embed, dtype=np.float32)
    proj_w = np.asarray(proj_w, dtype=np.float32)
    proj_b = np.asarray(proj_b, dtype=np.float32)

    Wt = proj_w[:, :D]           # [128, 128]
    Wc = proj_w[:, D:D + 8]      # [128, 8]
    Ws = proj_w[:, D + 8]        # [128]

    P = trait_embed @ Wt.T                      # [V, 128] projected table
    P_lo = np.ascontiguousarray(P[:SPLIT]).astype(bf16)
    P_hi = np.concatenate([np.zeros((1, D), np.float32), P[SPLIT:]], 0).astype(bf16)
    Pc = cat_embed @ Wc.T                       # [32, 128]
    MqT = np.concatenate([Pc, Ws[None, :], proj_b[None, :]], 0).astype(np.float32)  # [34,128]

    w = scores * (ids != 0)                     # [N, T]
    node_idx = np.repeat(np.arange(N, dtype=np.int64), T)
    hist = np.bincount(node_idx * 32 + cats.reshape(-1), weights=w.reshape(-1),
                       minlength=N * 32).reshape(N, 32)
    sws = (w * scores).sum(1)
    sw = w.sum(1)
    inv = (1.0 / np.maximum(sw, 1e-8)).astype(np.float32)
    q = np.concatenate([hist, sws[:, None], sw[:, None]], 1).astype(np.float32)  # [N,34]
    q *= inv[:, None]                           # fold the 1/sum_w into q
    wi = w * inv[:, None]                       # folded per-token weights

    # ---- structural chunk counts: max over cores per (page, group, table) ----
    lo_cnt = (ids < SPLIT).sum(1)               # per node (incl. id==0 pads -> lo)
    hi_cnt = T - lo_cnt
    ngroups = [math.ceil(_page_nodes(p) / GROUP) for p in range(NPAGES)]
    nchunks = []
    for p in range(NPAGES):
        per_t = [[], []]
        for g in range(ngroups[p]):
            n0 = p * PAGE + g * GROUP
            n1 = min(p * PAGE + _page_nodes(p), n0 + GROUP)
            best = [0, 0]
            for c in range(NCORES):
                sl = slice(c * NPC + n0, c * NPC + n1)
                idg = ids[sl].reshape(-1)
                ulo = len(np.unique(idg[idg < SPLIT]))
                uhi = len(np.unique(idg[idg >= SPLIT]))
                best[0] = max(best[0], math.ceil(ulo / 128))
                best[1] = max(best[1], math.ceil(uhi / 128))
            per_t[0].append(int(best[0]))
            per_t[1].append(int(best[1]))
        nchunks.append(per_t)

    # global chunk layout: page -> table -> group -> chunks
    chunk_group = []   # group index within page, per global chunk
    calls = []         # per page: list of [table, chunk0, nch, queue]
    last_chunk_of_page = []
    cbase = 0
    flat_calls = []
    for p in range(NPAGES):
        page_calls = []
        for t in (0, 1):
            run_chunks = sum(nchunks[p][t])
            for g in range(ngroups[p]):
                chunk_group.extend([g] * nchunks[p][t][g])
            if p == 0 and t == 0:
                # tiny warm-up calls, one per queue: desc-gen starts as soon
                # as a small idx slice lands while the bulk idx loads stream
                sizes = [2] * NQUEUES + _split_equal(run_chunks - 2 * NQUEUES,
                                                     CALL_BIG)
            else:
                sizes = _split_equal(run_chunks, CALL_BIG)
            done = 0
            for nch in sizes:
                page_calls.append([t, cbase + done, nch, 0])
                flat_calls.append(page_calls[-1])
                done += nch
            cbase += run_chunks
        calls.append(page_calls)
        last_chunk_of_page.append(cbase - 1)
    total_chunks = cbase

    # greedy least-loaded queue assignment (issue order fixed)
    qload = [0.0] * NQUEUES
    for call in flat_calls:
        dur = call[2] * 128 * NS_PER_IDX + NS_PER_CALL
        qmin = int(np.argmin(qload))
        call[3] = qmin
        qload[qmin] += dur

    meta = dict(calls=calls, chunk_group=chunk_group,
                last_chunk_of_page=last_chunk_of_page,
                total_chunks=total_chunks, ngroups=ngroups, perm=perm,
                qload=qload)

    # ---- per-core stream arrays ----
    in_maps = []
    for c in range(NCORES):
        idx_flat = np.zeros(total_chunks * 128, np.int16)
        tok_row, tok_col, tok_w = [], [], []
        cb = 0
        for p in range(NPAGES):
            for t in (0, 1):
                for g in range(ngroups[p]):
                    n0 = p * PAGE + g * GROUP
                    n1 = min(p * PAGE + _page_nodes(p), n0 + GROUP)
                    sl = slice(c * NPC + n0, c * NPC + n1)
                    idg = ids[sl]          # [ng, T]
                    wg = wi[sl]
                    m = (idg < SPLIT) if t == 0 else (idg >= SPLIT)
                    rows, cols = np.nonzero(m)
                    vals = idg[rows, cols]
                    if t == 1:
                        vals = vals - SPLIT + 1
                    # dedup: gather each unique id once; W carries multiplicity
                    uniq, inv_u = np.unique(vals, return_inverse=True)
                    k = len(uniq)
                    nch = nchunks[p][t][g]
                    off = cb * 128
                    idx_flat[off:off + k] = uniq.astype(np.int16)
                    tok_row.append(off + inv_u)
                    tok_col.append(rows)
                    tok_w.append(wg[rows, cols])
                    cb += nch
        assert cb == total_chunks

        # idx pack: per call [16, cols] tiled to 128 partitions
        idx_cols = np.empty((128, total_chunks * 8), np.int16)
        for page_calls in calls:
            for (_, c0, nch, _q) in page_calls:
                fl = idx_flat[c0 * 128:(c0 + nch) * 128]
                blk = fl.reshape(-1, 16).T           # [16, nch*8]
                idx_cols[:, c0 * 8:(c0 + nch) * 8] = np.tile(blk, (8, 1))

        # host-built W stream: [128, TC*64] fp8, chunk-major, with dup
        # tokens accumulated onto their unique gathered row
        wmat = np.zeros((total_chunks, 128, GROUP), np.float32)
        trow = np.concatenate(tok_row)
        tcol = np.concatenate(tok_col)
        tw = np.concatenate(tok_w)
        np.add.at(wmat, (trow // 128, trow % 128, tcol), tw)
        wmat = np.ascontiguousarray(
            wmat.transpose(1, 0, 2).reshape(128, total_chunks * GROUP)).astype(f8)

        qc = np.zeros((NPAGES * PAGE, 34), np.float32)
        qc[:NPC] = q[c * NPC:(c + 1) * NPC]
        q_arr = np.ascontiguousarray(qc.T)           # [34, NPAGES*PAGE]

        in_maps.append({
            "p_lo": np.asarray(P_lo), "p_hi": np.asarray(P_hi),
            "idxs": idx_cols, "wmat": wmat, "q": q_arr, "mqt": MqT,
        })
    return meta, in_maps


def _build(meta):
    f32, bft, i16 = mybir.dt.float32, mybir.dt.bfloat16, mybir.dt.int16
    TC = meta["total_chunks"]
    calls, chunk_group = meta["calls"], meta["chunk_group"]
    last_of = meta["last_chunk_of_page"]

    nc = bacc.Bacc("TRN2", target_bir_lowering=False, debug=False,
                   num_swdge_queues=NQUEUES)
    p_lo_d = nc.dram_tensor("p_lo", [SPLIT, D], bft, kind="ExternalInput")
    p_hi_d = nc.dram_tensor("p_hi", [V - SPLIT + 1, D], bft, kind="ExternalInput")
    idx_d = nc.dram_tensor("idxs", [128, TC * 8], i16, kind="ExternalInput")
    w_d = nc.dram_tensor("wmat", [128, TC * GROUP], mybir.dt.float8e4,
                         kind="ExternalInput")
    q_d = nc.dram_tensor("q", [34, NPAGES * PAGE], f32, kind="ExternalInput")
    mqt_d = nc.dram_tensor("mqt", [34, D], f32, kind="ExternalInput")
    out_d = nc.dram_tensor("out", [128, NPAGES * PAGE], f32, kind="ExternalOutput")

    with tile.TileContext(nc) as tc:
        with (
            tc.tile_pool(name="const", bufs=1) as const,
            tc.tile_pool(name="gpb", bufs=8) as gpb,
            tc.tile_pool(name="wb", bufs=6) as wbp,
            tc.tile_pool(name="nsb", bufs=2) as nsb,
            tc.tile_pool(name="psm", bufs=2, space="PSUM") as psm,
        ):
            idx_sb = const.tile([128, TC * 8], i16)
            q_sb = const.tile([34, NPAGES * PAGE], f32)
            mqt_sb = const.tile([34, D], f32)

            # idx loads: the warm-up calls' slice alone (fast ramp), then the
            # rest of page 0, then per page.
            nc.gpsimd.load_library(mlp)
            nch_0 = sum(cl[2] for cl in calls[0][:NQUEUES])
            nc.sync.dma_start(idx_sb[:, :nch_0 * 8], idx_d[:, :nch_0 * 8])
            cprev = nch_0
            for p in range(NPAGES):
                cend = last_of[p] + 1
                if cend > cprev:
                    nc.sync.dma_start(idx_sb[:, cprev * 8:cend * 8],
                                      idx_d[:, cprev * 8:cend * 8])
                cprev = cend
            nc.sync.dma_start(q_sb[:], q_d[:])
            nc.sync.dma_start(mqt_sb[:], mqt_d[:])

            size_regs = {}
            for p in range(NPAGES):
                ps = psm.tile([128, PAGE], mybir.dt.float32)
                nc.tensor.matmul(ps[:], mqt_sb[:],
                                 q_sb[:, p * PAGE:(p + 1) * PAGE],
                                 start=True, stop=False)
                for (t, c0, nch, qn) in calls[p]:
                    src = p_lo_d if t == 0 else p_hi_d
                    g_t = gpb.tile([128, CALL_BIG, D], bft, tag="gb")
                    if nch * 128 not in size_regs:
                        size_regs[nch * 128] = nc.gpsimd.to_reg(nch * 128)
                    nc.gpsimd.dma_gather(
                        g_t[:, :nch, :], src[:],
                        idx_sb[:, c0 * 8:(c0 + nch) * 8],
                        nch * 128, size_regs[nch * 128], D, queue_num=qn,
                        single_packet=False)
                    w_t = wbp.tile([128, CALL_BIG * GROUP], mybir.dt.float8e4,
                                   tag="wb")
                    nc.sync.dma_start(w_t[:, :nch * GROUP],
                                      w_d[:, c0 * GROUP:(c0 + nch) * GROUP])
                    for k in range(nch):
                        c = c0 + k
                        g = chunk_group[c]
                        nc.tensor.matmul(
                            ps[:, g * GROUP:(g + 1) * GROUP],
                            g_t[:, k, :], w_t[:, k * GROUP:(k + 1) * GROUP],
                            start=False, stop=(c == last_of[p]))

                num_sb = nsb.tile([128, PAGE], mybir.dt.float32)
                nc.vector.tensor_copy(num_sb[:], ps[:])
                nc.sync.dma_start(out_d[:, p * PAGE:(p + 1) * PAGE], num_sb[:])

    nc.compile()
    return nc


TRACE = False       # test harness can flip this for profiling
LAST_RESULT = None  # BassKernelResults of the most recent run


def kernel(**inputs) -> np.ndarray:
    global LAST_RESULT
    meta, in_maps = _prep(**inputs)
    nc = _build(meta)
    res = run_bass_kernel_spmd(nc, in_maps, list(range(NCORES)), trace=TRACE)
    LAST_RESULT = res
    outs = [np.asarray(r["out"]).T[:NPC] for r in res.results]
    rows = np.concatenate(outs, 0).astype(np.float32)
    full = np.empty_like(rows)
    full[meta["perm"]] = rows        # undo the node-balancing permutation
    return full


if __name__ == "__main__":
    rng = np.random.default_rng(0)
    demo = dict(
        token_ids=rng.integers(0, V, (N, T)),
        scores=rng.random((N, T), dtype=np.float32),
        cat_ids=rng.integers(0, 32, (N, T)),
        trait_embed=(rng.standard_normal((V, D)).astype(np.float32) * 0.02),
        cat_embed=(rng.standard_normal((32, 8)).astype(np.float32) * 0.02),
        proj_w=rng.standard_normal((D, D + 9)).astype(np.float32) / np.sqrt(137),
        proj_b=np.zeros(D, np.float32),
    )
    demo["trait_embed"][0] = 0
    out = kernel(**demo)
    print(out.shape, out.dtype)
